# revision 4
# baseline (speedup 1.0000x reference)
"""Connectome kernel (segment-mean -> Pearson Gram) for 8 TRN2 NeuronCores.

Strategy (pure data parallel, 2 samples per core):
  - Host marshalling: fold mask into parc (background label -1), keep
    ONLY contributing pixels (~50% of V), sort them by ROI block
    (B: rois 128-199 first, then A: rois 0-127), pad each block to a
    multiple of 128, and lay x out pixel-major, span-blocked, already
    cast to fp16. This halves HBM bytes twice over vs fp32 full-V
    (fp16 + used-pixels-only) and removes the on-device cast.
  - Device: 2.4MB fp16 span loads, each split half/half across the two
    HWDGE rings (scalar+sync queues).
  - Segment-sum as matmul: pixels pre-sorted by ROI block, so each
    128-pixel chunk needs ONE matmul: acc += onehot.T @ xT, fp16
    operands, fp32 PSUM accumulation; onehot (128,128) built per chunk
    from the label vector (is_equal vs iota), alternating between the
    gpsimd and vector engines so neither serializes the PE.
  - Epilogue (fp32): block B finishes first (B chunks run first), so
    its scale/demean/normalize chain is issued BEFORE anything that
    depends on block A and overlaps the A matmul phase. Tail work after
    the last matmul: A-block normalize, PE transposes, Gram matmuls,
    (2,200,200) conn DMA out.
  - Host: concat cores, extract upper triangle -> (16, 19900).
"""
import sys

sys.path.insert(0, "/opt/trn_rl_repo")

import numpy as np

import concourse.bass as bass
import concourse.tile as tile
from concourse import bacc, mybir
from concourse.bass_utils import run_bass_kernel_spmd

F32 = mybir.dt.float32
F16 = mybir.dt.float16

N, T, H, W = 16, 200, 144, 320
V = H * W                      # 46080
R = 200                        # ROIs
RP = 256                       # padded ROI dim (two 128-wide blocks)
NCORES = 8
SPB = N // NCORES              # samples per core = 2
ROWS = SPB * T                 # 400
NV = 24                        # 128-pixel chunks per load span
EPS = 1e-8

_cached = {}


def _build_program(cb, ca, nspan):
    """cb/ca: number of 128-pixel chunks in ROI block B/A (B first)."""
    nc = bacc.Bacc("TRN2", target_bir_lowering=False, debug=False)
    ntot = cb + ca

    x_d = nc.declare_dram_parameter("x", [nspan, 128, NV * ROWS], F16,
                                    isOutput=False)
    parc_d = nc.declare_dram_parameter("parcv", [128, ntot], F32, isOutput=False)
    iota_d = nc.declare_dram_parameter("iota", [128, RP], F16, isOutput=False)
    invca_d = nc.declare_dram_parameter("invca", [128, 1], F32, isOutput=False)
    invcb_d = nc.declare_dram_parameter("invcb", [72, 1], F32, isOutput=False)
    i128_d = nc.declare_dram_parameter("i128", [128, 128], F32, isOutput=False)
    i72_d = nc.declare_dram_parameter("i72", [72, 72], F32, isOutput=False)
    out_d = nc.declare_dram_parameter("conn", [SPB, R, R], F32, isOutput=True)

    with tile.TileContext(nc) as tc:
        with tc.tile_pool(name="consts", bufs=1) as consts, \
             tc.tile_pool(name="loads", bufs=3) as loads, \
             tc.tile_pool(name="ohp", bufs=8) as ohp, \
             tc.tile_pool(name="epi", bufs=1) as epi, \
             tc.tile_pool(name="psum", bufs=1, space="PSUM") as psum:

            parc_s = consts.tile([128, ntot], F32)
            iota_s = consts.tile([128, RP], F16)
            invca_s = consts.tile([128, 1], F32)
            invcb_s = consts.tile([72, 1], F32)
            i128_s = consts.tile([128, 128], F32)
            i72_s = consts.tile([72, 72], F32)
            nc.sync.dma_start(parc_s[:], parc_d[:])
            nc.sync.dma_start(iota_s[:], iota_d[:])
            nc.sync.dma_start(invca_s[:], invca_d[:])
            nc.sync.dma_start(invcb_s[:], invcb_d[:])
            nc.sync.dma_start(i128_s[:], i128_d[:])
            nc.sync.dma_start(i72_s[:], i72_d[:])

            acc_a = psum.tile([128, ROWS], F32, tag="acc_a", bufs=1)
            acc_b = psum.tile([128, ROWS], F32, tag="acc_b", bufs=1)

            HALF = NV // 2
            with nc.named_scope("main"):
                for sp in range(nspan):
                    ld = loads.tile([128, NV, ROWS], F16, tag="ld", bufs=3,
                                    name=f"ld_{sp}")
                    flat = ld[:].rearrange("p b r -> p (b r)")
                    nc.scalar.dma_start(flat[:, :HALF * ROWS],
                                        x_d[sp, :, :HALF * ROWS])
                    nc.sync.dma_start(flat[:, HALF * ROWS:],
                                      x_d[sp, :, HALF * ROWS:])

                    for b in range(NV):
                        cc = sp * NV + b
                        if cc >= ntot:
                            continue  # tail padding chunk: nothing to do
                        in_b = cc < cb
                        oh = ohp.tile([128, 128], F16, tag="oh", bufs=8,
                                      name=f"oh_{cc}")
                        blk = iota_s[:, 128:256] if in_b else iota_s[:, 0:128]
                        nc.vector.tensor_scalar(oh[:], blk,
                                           parc_s[:, cc:cc + 1], None,
                                           op0=mybir.AluOpType.is_equal)
                        acc = acc_b if in_b else acc_a
                        first = (cc == 0) if in_b else (cc == cb)
                        last = (cc == cb - 1) if in_b else (cc == ntot - 1)
                        nc.tensor.matmul(acc[:], oh[:], ld[:, b, :],
                                         start=first, stop=last)

            with nc.named_scope("epilogue"):
                # ---- block B chain first: overlaps the A matmul phase ----
                roiN = {}

                def norm_chain(blk, rt, P):
                    for s in range(SPB):
                        sl = bass.ts(s, T)
                        mean = epi.tile([P, 1], F32, name=f"mean_{blk}{s}",
                                        tag=f"mean_{blk}{s}")
                        nc.vector.tensor_reduce(mean[:], rt[:, sl],
                                                axis=mybir.AxisListType.X,
                                                op=mybir.AluOpType.add)
                        nc.vector.tensor_scalar_mul(mean[:], mean[:], 1.0 / T)
                        rc = epi.tile([P, T], F32, name=f"rc_{blk}{s}",
                                      tag=f"rc_{blk}{s}")
                        nc.vector.tensor_scalar(rc[:], rt[:, sl], mean[:], None,
                                                op0=mybir.AluOpType.subtract)
                        sq = epi.tile([P, T], F32, name=f"sq_{blk}{s}",
                                      tag=f"sq_{blk}{s}")
                        ss = epi.tile([P, 1], F32, name=f"ss_{blk}{s}",
                                      tag=f"ss_{blk}{s}")
                        nc.vector.scalar_tensor_tensor(
                            sq[:], rc[:], 1.0, rc[:],
                            op0=mybir.AluOpType.mult, op1=mybir.AluOpType.mult,
                            accum_out=ss[:])
                        nc.scalar.sqrt(ss[:], ss[:])
                        nc.vector.tensor_scalar_add(ss[:], ss[:], EPS)
                        nc.vector.reciprocal(ss[:], ss[:])
                        rn = epi.tile([P, T], F32, name=f"rn_{blk}{s}",
                                      tag=f"rn_{blk}{s}")
                        nc.vector.tensor_scalar_mul(rn[:], rc[:], ss[:])
                        roiN[(blk, s)] = rn

                # scale by 1/count straight out of PSUM (b rows 72+ junk)
                roi_b = epi.tile([72, ROWS], F32)
                nc.vector.tensor_scalar_mul(roi_b[:], acc_b[0:72, :], invcb_s[:])
                norm_chain("b", roi_b, 72)

                # ---- block A chain: tail after last matmul ----
                roi_a = epi.tile([128, ROWS], F32)
                nc.vector.tensor_scalar_mul(roi_a[:], acc_a[:], invca_s[:])
                norm_chain("a", roi_a, 128)

                for s in range(SPB):
                    # transpose roiN -> (t, r) on PE
                    trA = psum.tile([128, R], F32, tag="trA", bufs=1,
                                    name=f"trA_{s}")
                    trB = psum.tile([72, R], F32, tag="trB", bufs=1,
                                    name=f"trB_{s}")
                    nc.tensor.transpose(trA[:, 0:128], roiN[("a", s)][:, 0:128],
                                        i128_s[:])
                    nc.tensor.transpose(trA[:, 128:200], roiN[("b", s)][:, 0:128],
                                        i72_s[:])
                    nc.tensor.transpose(trB[:, 0:128], roiN[("a", s)][:, 128:200],
                                        i128_s[:])
                    nc.tensor.transpose(trB[:, 128:200], roiN[("b", s)][:, 128:200],
                                        i72_s[:])
                    trA_sb = epi.tile([128, R], F32, name=f"trAs_{s}",
                                      tag=f"trAs_{s}")
                    trB_sb = epi.tile([72, R], F32, name=f"trBs_{s}",
                                      tag=f"trBs_{s}")
                    nc.vector.tensor_copy(trA_sb[:], trA[:])
                    nc.vector.tensor_copy(trB_sb[:], trB[:])

                    # Gram: conn = roiN_t.T @ roiN_t  (contraction over t)
                    cA = psum.tile([128, R], F32, tag="cA", bufs=1, name=f"cA_{s}")
                    cB = psum.tile([72, R], F32, tag="cB", bufs=1, name=f"cB_{s}")
                    nc.tensor.matmul(cA[:], trA_sb[:, 0:128], trA_sb[:],
                                     start=True, stop=False)
                    nc.tensor.matmul(cA[:], trB_sb[:, 0:128], trB_sb[:],
                                     start=False, stop=True)
                    nc.tensor.matmul(cB[:], trA_sb[:, 128:200], trA_sb[:],
                                     start=True, stop=False)
                    nc.tensor.matmul(cB[:], trB_sb[:, 128:200], trB_sb[:],
                                     start=False, stop=True)
                    cA_sb = epi.tile([128, R], F32, name=f"cAs_{s}", tag=f"cAs_{s}")
                    cB_sb = epi.tile([72, R], F32, name=f"cBs_{s}", tag=f"cBs_{s}")
                    nc.vector.tensor_copy(cA_sb[:], cA[:])
                    nc.vector.tensor_copy(cB_sb[:], cB[:])
                    nc.sync.dma_start(out_d[s, 0:128, :], cA_sb[:])
                    nc.scalar.dma_start(out_d[s, 128:200, :], cB_sb[:])

    nc.compile()
    return nc


def _get_program():
    if "nc" not in _cached:
        cb, ca, nspan = _cached["geom"]
        _cached["nc"] = _build_program(cb, ca, nspan)
    return _cached["nc"]


def marshal_inputs(x, parc, mask):
    """Host-side prep: gather used pixels, sort by ROI block, fp16 cast."""
    x = np.asarray(x)
    parc_eff = np.where(np.asarray(mask), np.asarray(parc), 0).reshape(V)
    counts = np.bincount(parc_eff.astype(np.int64), minlength=R + 1).astype(np.float32)
    inv = np.float32(1.0) / counts[1:]                      # (200,)
    lab = parc_eff.astype(np.int64) - 1                     # -1 for background

    idx_b = np.nonzero(lab >= 128)[0]
    idx_a = np.nonzero((lab >= 0) & (lab < 128))[0]
    cb = (len(idx_b) + 127) // 128
    ca = (len(idx_a) + 127) // 128
    nspan = (cb + ca + NV - 1) // NV
    ctot_pad = nspan * NV
    _cached["geom"] = (cb, ca, nspan)

    # padded pixel permutation; pads point at pixel 0 with label -1
    perm = np.zeros(ctot_pad * 128, dtype=np.int64)
    labp = np.full(ctot_pad * 128, -1.0, dtype=np.float32)
    perm[:len(idx_b)] = idx_b
    labp[:len(idx_b)] = lab[idx_b]
    off = cb * 128
    perm[off:off + len(idx_a)] = idx_a
    labp[off:off + len(idx_a)] = lab[idx_a]

    parcv = labp[:(cb + ca) * 128].reshape(cb + ca, 128).T.copy()  # (128, ntot)
    iota = np.broadcast_to(np.arange(RP, dtype=np.float16), (128, RP)).copy()
    invca = inv[0:128].reshape(128, 1).copy()
    invcb = inv[128:200].reshape(72, 1).copy()
    i128 = np.eye(128, dtype=np.float32)
    i72 = np.eye(72, dtype=np.float32)

    # (N, T, V) -> gather used pixels -> per-core (nspan, 128, NV*SPB*T)
    xf = x.reshape(N, T, V)
    xg = xf[:, :, perm]                                       # (N, T, ctot*128)
    xr = xg.reshape(NCORES, SPB, T, nspan, NV, 128)
    xh = np.ascontiguousarray(xr.transpose(0, 3, 5, 4, 1, 2),
                              dtype=np.float16)               # (8, sp, p, b, s, t)
    xh = xh.reshape(NCORES, nspan, 128, NV * ROWS)

    in_maps = []
    for c in range(NCORES):
        in_maps.append({
            "x": xh[c], "parcv": parcv, "iota": iota,
            "invca": invca, "invcb": invcb, "i128": i128, "i72": i72,
        })
    return in_maps


def kernel(x, parc, mask):
    in_maps = marshal_inputs(x, parc, mask)
    nc = _get_program()
    res = run_bass_kernel_spmd(nc, in_maps, core_ids=list(range(NCORES)))
    conn = np.concatenate([r["conn"] for r in res.results], axis=0)  # (16,200,200)
    row, col = np.triu_indices(R, k=1)
    return np.ascontiguousarray(conn[:, row, col]).astype(np.float32)
